# revision 1
# baseline (speedup 1.0000x reference)
"""Trainium2 Bass kernel for e3nn-style GNN message passing convolution.

Strategy (8 cores, no collectives):
 - Shard edges by DESTINATION node range: core k owns nodes [1250k, 1250(k+1))
   and all edges pointing into that range. Each core computes its own output
   slice; host concatenates. No all-reduce needed.
 - Within a core, edges are grouped by 128-node destination blocks (host-side
   bucketing). Per block, a PSUM tile [128 nodes, 64] accumulates
   Sel.T @ edge_feat scatter matmuls across that block's edge tiles.
 - Edge MLP runs on TensorE in float32r (full fp32 precision at bf16 rate).
 - Tensor product runs on VectorE edge-major with (v,u)-inner layouts
   (W2 columns host-permuted) so bf16 2x packing applies; path constants
   (ALPHA, C3, 1/sqrt(deg)) are folded into the z-vectors.
"""
import math
from contextlib import ExitStack

import numpy as np

import concourse.bass as bass
import concourse.tile as tile
from concourse import bacc, mybir
from concourse import bass_utils

N_NODES = 10000
N_EDGES = 160000
MUL = 16
DIM_EMB = 64
HID = 256
NCORES = 8
NPC = N_NODES // NCORES          # 1250 nodes per core
P = 128
NBLK = math.ceil(NPC / P)        # 10 node blocks per core
C3 = 1.0 / math.sqrt(3.0)
ALPHA = 1.0 / math.sqrt(2 * MUL)
GN = 1.0 / math.sqrt(N_EDGES / N_NODES)   # segment-sum normalization

F32 = mybir.dt.float32
F32R = mybir.dt.float32r
BF16 = mybir.dt.bfloat16
I32 = mybir.dt.int32

_CACHE = {}


def _build(tiles_per_block, tp_dtype=BF16):
    """Build the Bass program for a fixed per-block tile schedule."""
    T = sum(tiles_per_block)           # total 128-edge tiles per core
    E = T * P
    nc = bacc.Bacc("TRN2", target_bir_lowering=False, debug=False,
                   num_devices=NCORES)

    embT_d = nc.dram_tensor("embT", [64, E], F32R, kind="ExternalInput").ap()
    attr_d = nc.dram_tensor("attr", [E, 4], F32, kind="ExternalInput").ap()
    srci_d = nc.dram_tensor("srci", [E, 1], I32, kind="ExternalInput").ap()
    dstr_d = nc.dram_tensor("dstr", [E, 1], F32, kind="ExternalInput").ap()
    nf_d = nc.dram_tensor("nf", [N_NODES, 64], F32, kind="ExternalInput").ap()
    w1_d = nc.dram_tensor("w1", [64, HID], F32R, kind="ExternalInput").ap()
    b1_d = nc.dram_tensor("b1", [HID, 1], F32, kind="ExternalInput").ap()
    w2_d = nc.dram_tensor("w2", [HID, 1024], F32R, kind="ExternalInput").ap()
    iota_d = nc.dram_tensor("iota", [P, P], F32, kind="ExternalInput").ap()
    out_d = nc.dram_tensor("out", [NBLK * P, 64], F32, kind="ExternalOutput").ap()

    # block id for each tile, and first/last flags
    tile_blk, first, last = [], [], []
    for b, nt in enumerate(tiles_per_block):
        for i in range(nt):
            tile_blk.append(b)
            first.append(i == 0)
            last.append(i == nt - 1)

    NG = T // 4  # groups of 4 subtiles (T is padded to a multiple of 4)

    with tile.TileContext(nc) as tc, ExitStack() as ctx:
        const = ctx.enter_context(tc.tile_pool(name="const", bufs=1))
        sbB = ctx.enter_context(tc.tile_pool(name="sbB", bufs=3))   # big streaming
        sbS = ctx.enter_context(tc.tile_pool(name="sbS", bufs=4))   # small per-subtile
        psH = ctx.enter_context(tc.tile_pool(name="psH", bufs=2, space="PSUM"))
        psW = ctx.enter_context(tc.tile_pool(name="psW", bufs=2, space="PSUM"))
        psO = ctx.enter_context(tc.tile_pool(name="psO", bufs=2, space="PSUM"))

        # constants
        w1_t = const.tile([64, HID], F32R)
        nc.sync.dma_start(w1_t[:], w1_d[:])
        b1_h = [const.tile([P, 1], F32, name=f"b1_{i}", tag=f"b1_{i}") for i in range(2)]
        w2_h = [const.tile([P, 1024], F32R, name=f"w2_{i}", tag=f"w2_{i}") for i in range(2)]
        for i in range(2):
            nc.sync.dma_start(b1_h[i][:], b1_d[i * P:(i + 1) * P, :])
            nc.sync.dma_start(w2_h[i][:], w2_d[i * P:(i + 1) * P, :])
        iota_t = const.tile([P, P], F32)
        nc.sync.dma_start(iota_t[:], iota_d[:])

        sc_ps = None
        cur_blk = -1
        for g in range(NG):
            e0 = g * 512
            # ---- load + W1 GEMM (feature-major, per 512-edge group)
            embT = sbB.tile([64, 512], F32R, tag="embT")
            nc.sync.dma_start(embT[:], embT_d[:, e0:e0 + 512])
            gT = [sbB.tile([P, 512], F32R, name=f"gT{hh}", tag=f"gT{hh}") for hh in range(2)]
            for hh in range(2):
                h_ps = psH.tile([P, 512], F32, space="PSUM", tag="hps")
                nc.tensor.matmul(h_ps[:], w1_t[:, hh * P:(hh + 1) * P], embT[:],
                                 start=True, stop=True)
                # silu(h + b1) -> gT  (bias per-partition)
                nc.scalar.activation(gT[hh][:], h_ps[:],
                                     mybir.ActivationFunctionType.Silu,
                                     bias=b1_h[hh][:])

            for s in range(4):
                t_idx = g * 4 + s
                se0 = e0 + s * P
                blk = tile_blk[t_idx]

                # ---- per-subtile loads
                attr = sbS.tile([P, 4], F32, tag="attr")
                nc.sync.dma_start(attr[:], attr_d[se0:se0 + P, :])
                srci = sbS.tile([P, 1], I32, tag="srci")
                nc.sync.dma_start(srci[:], srci_d[se0:se0 + P, :])
                dstr = sbS.tile([P, 1], F32, tag="dstr")
                nc.sync.dma_start(dstr[:], dstr_d[se0:se0 + P, :])
                src = sbS.tile([P, 64], F32, tag="src")
                nc.gpsimd.indirect_dma_start(
                    out=src[:], out_offset=None, in_=nf_d[:],
                    in_offset=bass.IndirectOffsetOnAxis(ap=srci[:, :1], axis=0))

                # ---- W2 GEMM: w [128e, 1024] in two 512-col chunks, drain to bf16
                wb = sbS.tile([P, 1024], tp_dtype, tag="wb")
                for ch in range(2):
                    w_ps = psW.tile([P, 512], F32, space="PSUM", tag="wps")
                    for kk in range(2):
                        nc.tensor.matmul(
                            w_ps[:],
                            gT[kk][:, s * P:(s + 1) * P],
                            w2_h[kk][:, ch * 512:(ch + 1) * 512],
                            start=(kk == 0), stop=(kk == 1))
                    nc.scalar.activation(wb[:, ch * 512:(ch + 1) * 512], w_ps[:],
                                         mybir.ActivationFunctionType.Copy)

                # ---- z-builds (fold ALPHA, C3, GN); all bf16 outputs
                s1 = src[:, 0:16]
                v1 = src[:, 16:64]           # (u, m) layout
                s2 = attr[:, 0:1]
                # z_ss = s1*s2*ALPHA*GN
                z2 = sbS.tile([P, 32], tp_dtype, tag="z2")  # [z_ss | z_vv]
                zss_f = sbS.tile([P, 16], F32, tag="zss_f")
                nc.vector.tensor_scalar(zss_f[:], s1, s2, ALPHA * GN,
                                        op0=mybir.AluOpType.mult,
                                        op1=mybir.AluOpType.mult)
                nc.vector.tensor_copy(z2[:, 0:16], zss_f[:])
                # p_vv = v1*v2 ; z_vv = sum_m * (ALPHA*C3*GN)
                pvv = sbS.tile([P, 48], F32, tag="pvv")
                nc.vector.tensor_mul(
                    pvv[:].rearrange("p (u m) -> p u m", m=3),
                    v1.rearrange("p (u m) -> p u m", m=3),
                    attr[:, None, 1:4].to_broadcast([P, 16, 3]))
                zvv_f = sbS.tile([P, 16], F32, tag="zvv_f")
                nc.vector.reduce_sum(zvv_f[:, :, None],
                                     pvv[:].rearrange("p (u m) -> p u m", m=3),
                                     axis=mybir.AxisListType.X)
                nc.vector.tensor_scalar_mul(z2[:, 16:32], zvv_f[:], ALPHA * C3 * GN)
                # zm[(u,m)] = v1 * s2 * (ALPHA*C3*GN)
                zm = sbS.tile([P, 48], tp_dtype, tag="zm")
                nc.vector.tensor_scalar(zm[:], v1, s2, ALPHA * C3 * GN,
                                        op0=mybir.AluOpType.mult,
                                        op1=mybir.AluOpType.mult)
                # s1b = s1 * (ALPHA*C3*GN) ; v2b = v2
                s1b = sbS.tile([P, 16], tp_dtype, tag="s1b")
                nc.vector.tensor_scalar_mul(s1b[:], s1, ALPHA * C3 * GN)

                # ---- products + reduces. W2 col layout: [ss|vv|sv|vs], each (v,u)
                t1 = sbS.tile([P, 512], tp_dtype, tag="t1")   # ss+vv products
                nc.vector.tensor_mul(
                    t1[:].rearrange("p (g v u) -> p g v u", g=2, u=16),
                    wb[:, 0:512].rearrange("p (g v u) -> p g v u", g=2, u=16),
                    z2[:, :, None].rearrange("p (g u) x -> p g x u", g=2)
                        .to_broadcast([P, 2, 16, 16]))
                r1 = sbS.tile([P, 32], F32, tag="r1")    # [r_ss | r_vv]
                nc.vector.reduce_sum(
                    r1[:].rearrange("p (g v) -> p g v", g=2)[:, :, :, None],
                    t1[:].rearrange("p (g v u) -> p g v u", g=2, u=16),
                    axis=mybir.AxisListType.X)
                # sv: t_sv = w_sv * s1b ; r_sv[v]
                t2 = sbS.tile([P, 256], tp_dtype, tag="t2")
                nc.vector.tensor_mul(
                    t2[:].rearrange("p (v u) -> p v u", u=16),
                    wb[:, 512:768].rearrange("p (v u) -> p v u", u=16),
                    s1b[:, None, :].to_broadcast([P, 16, 16]))
                rsv = sbS.tile([P, 16], F32, tag="rsv")
                nc.vector.reduce_sum(rsv[:, :, None],
                                     t2[:].rearrange("p (v u) -> p v u", u=16),
                                     axis=mybir.AxisListType.X)
                # vs: t_vs[(m,v,u)] = w_vs[(v,u)] * zm[(u,m)] ; r_vs[(m,v)]
                t3 = sbS.tile([P, 768], tp_dtype, tag="t3")
                nc.vector.tensor_mul(
                    t3[:].rearrange("p (m v u) -> p m v u", m=3, u=16),
                    wb[:, None, 768:1024].rearrange("p x (v u) -> p x v u", u=16)
                        .to_broadcast([P, 3, 16, 16]),
                    zm[:].rearrange("p (u m) -> p m u", m=3)[:, :, None, :]
                        .to_broadcast([P, 3, 16, 16]))
                rvs = sbS.tile([P, 48], F32, tag="rvs")  # (m, v)
                nc.vector.reduce_sum(
                    rvs[:].rearrange("p (m v) -> p m v", m=3)[:, :, :, None],
                    t3[:].rearrange("p (m v u) -> p m v u", m=3, u=16),
                    axis=mybir.AxisListType.X)

                # ---- assemble edge_feat [P, 64]: [:16]=r_ss+r_vv, [16+3v+m]=r_vs+v2*r_sv
                feat = sbS.tile([P, 64], tp_dtype, tag="feat")
                nc.vector.tensor_add(feat[:, 0:16], r1[:, 0:16], r1[:, 16:32])
                sv3 = sbS.tile([P, 48], F32, tag="sv3")  # (m, v)
                for m in range(3):
                    nc.vector.tensor_scalar_mul(sv3[:, m * 16:(m + 1) * 16],
                                                rsv[:], attr[:, m + 1:m + 2])
                nc.vector.tensor_add(
                    feat[:, 16:64].rearrange("p (v m) -> p m v", m=3),
                    rvs[:].rearrange("p (m v) -> p m v", m=3),
                    sv3[:].rearrange("p (m v) -> p m v", m=3))

                # ---- scatter: Sel = is_equal(dst_rel, iota) ; psum += Sel.T @ feat
                sel = sbS.tile([P, P], tp_dtype, tag="sel")
                nc.vector.tensor_tensor(out=sel[:],
                                        in0=dstr[:].to_broadcast([P, P]),
                                        in1=iota_t[:],
                                        op=mybir.AluOpType.is_equal)
                if first[t_idx]:
                    sc_ps = psO.tile([P, 64], F32, space="PSUM", tag="scps")
                    cur_blk = blk
                assert cur_blk == blk
                nc.tensor.matmul(sc_ps[:], sel[:], feat[:],
                                 start=first[t_idx], stop=last[t_idx])
                if last[t_idx]:
                    outb = sbS.tile([P, 64], F32, tag="outb")
                    nc.scalar.activation(outb[:], sc_ps[:],
                                         mybir.ActivationFunctionType.Copy)
                    nc.sync.dma_start(out_d[blk * P:(blk + 1) * P, :], outb[:])

    nc.finalize()
    return nc


def _prep(inputs):
    """Host-side sharding: bucket edges by (core, node-block), pad to a
    uniform per-block tile grid, build per-core input maps."""
    nf = np.ascontiguousarray(inputs["node_features"], dtype=np.float32)
    esrc = inputs["edge_src"].astype(np.int64)
    edst = inputs["edge_dst"].astype(np.int64)
    eattr = np.asarray(inputs["edge_attr"], dtype=np.float32)
    eemb = np.asarray(inputs["edge_embedding"], dtype=np.float32)

    core = edst // NPC
    blk = (edst - core * NPC) // P
    key = core * NBLK + blk
    order = np.argsort(key, kind="stable")
    key_s = key[order]

    counts = np.bincount(key_s, minlength=NCORES * NBLK).reshape(NCORES, NBLK)
    tiles_per_block = [int(math.ceil(max(counts[:, b].max(), 1) / P))
                       for b in range(NBLK)]
    # pad total to a multiple of 4 (512-edge groups)
    while sum(tiles_per_block) % 4:
        tiles_per_block[-1] += 1
    T = sum(tiles_per_block)
    E = T * P
    starts = np.concatenate([[0], np.cumsum([t * P for t in tiles_per_block])])

    in_maps = []
    boundaries = np.searchsorted(key_s, np.arange(NCORES * NBLK + 1))
    for k in range(NCORES):
        embT = np.zeros((64, E), np.float32)
        attr = np.zeros((E, 4), np.float32)
        srci = np.zeros((E, 1), np.int32)
        dstr = np.full((E, 1), 999.0, np.float32)
        for b in range(NBLK):
            lo, hi = boundaries[k * NBLK + b], boundaries[k * NBLK + b + 1]
            n = hi - lo
            sl = order[lo:hi]
            o = int(starts[b])
            embT[:, o:o + n] = eemb[sl].T
            attr[o:o + n] = eattr[sl]
            srci[o:o + n, 0] = esrc[sl]
            dstr[o:o + n, 0] = (edst[sl] - k * NPC - b * P).astype(np.float32)
        in_maps.append(dict(embT=embT, attr=attr, srci=srci, dstr=dstr))
    return tiles_per_block, in_maps


def _w2_perm():
    """Permutation of W2's 1024 columns: (path,u,v) -> [ss|vv|sv|vs] each (v,u)."""
    idx = np.arange(1024).reshape(4, MUL, MUL)  # (path, u, v)
    blocks = [idx[0].T, idx[3].T, idx[2].T, idx[1].T]  # ss, vv, sv, vs as (v,u)
    return np.concatenate([b.reshape(-1) for b in blocks])


def kernel(**inputs):
    tiles_per_block, in_maps = _prep(inputs)
    key = tuple(tiles_per_block)
    if key not in _CACHE:
        _CACHE[key] = _build(tiles_per_block)
    nc = _CACHE[key]

    W1 = np.ascontiguousarray(inputs["W1"], np.float32)
    b1 = np.ascontiguousarray(inputs["b1"], np.float32).reshape(HID, 1)
    W2p = np.ascontiguousarray(
        np.asarray(inputs["W2"], np.float32)[:, _w2_perm()])
    nf = np.ascontiguousarray(inputs["node_features"], np.float32)
    iota = np.tile(np.arange(P, dtype=np.float32)[None, :], (P, 1))
    assert not np.any(inputs["b2"]), "b2 != 0 unsupported by this build"

    for m in in_maps:
        m.update(nf=nf, w1=W1, b1=b1, w2=W2p, iota=iota)

    res = bass_utils.run_bass_kernel_spmd(nc, in_maps,
                                          core_ids=list(range(NCORES)))
    out = np.empty((N_NODES, 64), np.float32)
    for k in range(NCORES):
        out[k * NPC:(k + 1) * NPC] = res.results[k]["out"][:NPC]
    return out



# revision 7
# speedup vs baseline: 1.9338x; 1.9338x over previous
"""Trainium2 Bass kernel for e3nn-style GNN message passing convolution.

Strategy (8 cores, no collectives):
 - Shard edges by DESTINATION node range: core k owns nodes [1250k, 1250(k+1))
   and all edges pointing into that range; host concatenates core outputs.
 - Within a core, edges are bucketed into 128-node destination blocks. Per
   block, a 4-bank PSUM tile [128 nodes, 2048] accumulates Sel.T @ t scatter
   matmuls over the block's edge tiles, where t[e, (c,u)] are the UNREDUCED
   tensor-product terms: the u-contraction of the TP is absorbed into a
   cheap per-block reduce after the scatter (instead of per-edge DVE
   reduces).
 - All GEMMs run in plain bf16 (emb, W1, W2, g, products, Sel).
 - Source-node features are gathered host-side (pure data movement) and
   streamed as a dense per-edge array; z-vectors are built on-chip with
   group-batched ops spread across Pool/Scalar/DVE.
"""
import math
from contextlib import ExitStack

import numpy as np

import concourse.bass as bass
import concourse.tile as tile
from concourse import bacc, mybir
from concourse import bass_utils

N_NODES = 10000
N_EDGES = 160000
MUL = 16
DIM_EMB = 64
HID = 256
NCORES = 8
NPC = N_NODES // NCORES          # 1250 nodes per core
P = 128
NBLK = math.ceil(NPC / P)        # 10 node blocks per core
C3 = 1.0 / math.sqrt(3.0)
ALPHA = 1.0 / math.sqrt(2 * MUL)
GN = 1.0 / math.sqrt(N_EDGES / N_NODES)   # segment-sum normalization
AGN = ALPHA * GN
ACGN = ALPHA * C3 * GN

F32 = mybir.dt.float32
BF16 = mybir.dt.bfloat16

_CACHE = {}


def _build(tiles_per_block):
    """Build the Bass program for a fixed per-block tile schedule."""
    T = sum(tiles_per_block)           # total 128-edge tiles per core
    NG = T // 4                        # 512-edge groups (T padded to 4)
    nc = bacc.Bacc("TRN2", target_bir_lowering=False, debug=False,
                   num_devices=NCORES)

    embT_d = nc.dram_tensor("embT", [64, T * P], BF16, kind="ExternalInput").ap()
    srcp_d = nc.dram_tensor("srcp", [NG * P, 256], BF16, kind="ExternalInput").ap()
    attrp_d = nc.dram_tensor("attrp", [NG * P, 32], F32, kind="ExternalInput").ap()
    w1_d = nc.dram_tensor("w1", [64, HID], BF16, kind="ExternalInput").ap()
    b1_d = nc.dram_tensor("b1", [HID, 1], F32, kind="ExternalInput").ap()
    w2_d = nc.dram_tensor("w2", [HID, 1024], BF16, kind="ExternalInput").ap()
    iota_d = nc.dram_tensor("iota", [P, P], F32, kind="ExternalInput").ap()
    out_d = nc.dram_tensor("out", [NBLK * P, 64], F32, kind="ExternalOutput").ap()

    # block id for each tile, and first/last flags
    tile_blk, first, last = [], [], []
    for b, nt in enumerate(tiles_per_block):
        for i in range(nt):
            tile_blk.append(b)
            first.append(i == 0)
            last.append(i == nt - 1)

    with tile.TileContext(nc) as tc, ExitStack() as ctx:
        const = ctx.enter_context(tc.tile_pool(name="const", bufs=1))
        sbB = ctx.enter_context(tc.tile_pool(name="sbB", bufs=3))   # group streams
        sbT = ctx.enter_context(tc.tile_pool(name="sbT", bufs=3))   # per-subtile
        sbO = ctx.enter_context(tc.tile_pool(name="sbO", bufs=2))   # block out
        psH = ctx.enter_context(tc.tile_pool(name="psH", bufs=2, space="PSUM"))
        psW = ctx.enter_context(tc.tile_pool(name="psW", bufs=2, space="PSUM"))
        psO = ctx.enter_context(tc.tile_pool(name="psO", bufs=1, space="PSUM"))

        # constants
        w1_t = const.tile([64, HID], BF16)
        nc.sync.dma_start(w1_t[:], w1_d[:])
        b1_h = [const.tile([P, 1], F32, name=f"b1_{i}", tag=f"b1_{i}") for i in range(2)]
        w2_h = [const.tile([P, 1024], BF16, name=f"w2_{i}", tag=f"w2_{i}") for i in range(2)]
        for i in range(2):
            nc.sync.dma_start(b1_h[i][:], b1_d[i * P:(i + 1) * P, :])
            nc.sync.dma_start(w2_h[i][:], w2_d[i * P:(i + 1) * P, :])
        iota_t = const.tile([P, P], F32)
        nc.sync.dma_start(iota_t[:], iota_d[:])

        sc_ps = None
        cur_blk = -1
        for g in range(NG):
            e0 = g * 512
            # ---- group loads
            embT = sbB.tile([64, 512], BF16, tag="embT")
            nc.sync.dma_start(embT[:], embT_d[:, e0:e0 + 512])
            srcp = sbB.tile([P, 256], BF16, tag="srcp")
            nc.sync.dma_start(srcp[:], srcp_d[g * P:(g + 1) * P, :])
            attrp = sbB.tile([P, 32], F32, tag="attrp")
            nc.sync.dma_start(attrp[:], attrp_d[g * P:(g + 1) * P, :])

            # ---- W1 GEMM + silu -> gT [h, e] bf16
            gT = [sbB.tile([P, 512], BF16, name=f"gT{hh}", tag=f"gT{hh}")
                  for hh in range(2)]
            for hh in range(2):
                h_ps = psH.tile([P, 512], F32, space="PSUM", tag="hps")
                nc.tensor.matmul(h_ps[:], w1_t[:, hh * P:(hh + 1) * P], embT[:],
                                 start=True, stop=True)
                nc.scalar.activation(gT[hh][:], h_ps[:],
                                     mybir.ActivationFunctionType.Silu,
                                     bias=b1_h[hh][:])

            # ---- group-batched z-builds
            # attrp cols per subtile s (stride 8): [s2, v2x, v2y, v2z, dstr, ...]
            attr4 = attrp[:].rearrange("p (s c) -> p s c", c=8)
            # zc [128, 32]: [0:4]=s2*AGN, [4:8]=s2*ACGN, [8:20]=v2*ACGN (s,m)
            zc = sbB.tile([P, 32], F32, tag="zc")
            nc.gpsimd.tensor_scalar_mul(zc[:, 0:4], attr4[:, :, 0], AGN)
            nc.gpsimd.tensor_scalar_mul(zc[:, 4:8], attr4[:, :, 0], ACGN)
            nc.gpsimd.tensor_scalar_mul(
                zc[:, 8:20].rearrange("p (s m) -> p s m", m=3),
                attr4[:, :, 1:4], ACGN)
            zc4 = zc[:].rearrange("p (x s) -> p x s", x=8)  # x* s-strided views
            # sel4 [128, (s,n)] bf16 = is_equal(dstr, iota)  (DVE: walrus
            # rejects is_equal on Pool)
            sel4 = sbB.tile([P, 512], BF16, tag="sel4")
            nc.vector.tensor_tensor(
                out=sel4[:].rearrange("p (s n) -> p s n", n=P),
                in0=attr4[:, :, 4:5].to_broadcast([P, 4, P]),
                in1=iota_t[:, None, :].to_broadcast([P, 4, P]),
                op=mybir.AluOpType.is_equal)

            # Z4 [128, (s,128)] bf16: per subtile [zss 16 | zvv 16 | z_sv(m,u) 48
            #                                      | z_vs(m,u) 48]
            Z4 = sbB.tile([P, 512], BF16, tag="Z4")
            Z4v = Z4[:].rearrange("p (s c) -> p s c", c=128)
            src4 = srcp[:].rearrange("p (s c) -> p s c", c=64)
            s1v = src4[:, :, 0:16]                              # [p, s, u]
            # zss = s1 * s2a
            nc.gpsimd.tensor_tensor(
                out=Z4v[:, :, 0:16], in0=s1v,
                in1=zc[:, 0:4][:, :, None].to_broadcast([P, 4, 16]),
                op=mybir.AluOpType.mult)
            # z_vs (zm) = v1[(u,m)->(m,u)] * s2b
            nc.gpsimd.tensor_tensor(
                out=Z4v[:, :, 80:128].rearrange("p s (m u) -> p s m u", u=16),
                in0=src4[:, :, 16:64].rearrange("p s (u m) -> p s m u", m=3),
                in1=zc[:, 4:8][:, :, None, None].to_broadcast([P, 4, 3, 16]),
                op=mybir.AluOpType.mult)
            # z_sv = s1 (x) v2c
            nc.gpsimd.tensor_tensor(
                out=Z4v[:, :, 32:80].rearrange("p s (m u) -> p s m u", u=16),
                in0=s1v[:, :, None, :].to_broadcast([P, 4, 3, 16]),
                in1=zc[:, 8:20].rearrange("p (s m) -> p s m", m=3)[:, :, :, None]
                    .to_broadcast([P, 4, 3, 16]),
                op=mybir.AluOpType.mult)
            # pvv = v1 * v2c (u,m); zvv = sum_m
            pvv = sbB.tile([P, 192], F32, tag="pvv")
            nc.gpsimd.tensor_tensor(
                out=pvv[:].rearrange("p (s u m) -> p s u m", s=4, m=3),
                in0=src4[:, :, 16:64].rearrange("p s (u m) -> p s u m", m=3),
                in1=zc[:, 8:20].rearrange("p (s m) -> p s m", m=3)[:, :, None, :]
                    .to_broadcast([P, 4, 16, 3]),
                op=mybir.AluOpType.mult)
            zvvf = sbB.tile([P, 64], F32, tag="zvvf")
            nc.vector.reduce_sum(
                zvvf[:].rearrange("p (s u) -> p s u", s=4)[:, :, :, None],
                pvv[:].rearrange("p (s u m) -> p s u m", s=4, m=3),
                axis=mybir.AxisListType.X)
            nc.vector.tensor_copy(
                Z4v[:, :, 16:32],
                zvvf[:].rearrange("p (s u) -> p s u", s=4))

            for s in range(4):
                t_idx = g * 4 + s
                blk = tile_blk[t_idx]

                # ---- W2 GEMM: w [128e, 1024] in two 512-col chunks
                wb = sbT.tile([P, 1024], BF16, tag="wb")
                for ch in range(2):
                    w_ps = psW.tile([P, 512], F32, space="PSUM", tag="wps")
                    for kk in range(2):
                        nc.tensor.matmul(
                            w_ps[:],
                            gT[kk][:, s * P:(s + 1) * P],
                            w2_h[kk][:, ch * 512:(ch + 1) * 512],
                            start=(kk == 0), stop=(kk == 1))
                    # Pool cannot read PSUM; drains on ScalarE
                    nc.scalar.activation(wb[:, ch * 512:(ch + 1) * 512], w_ps[:],
                                         mybir.ActivationFunctionType.Copy)

                # ---- products -> t [128, 2048] bf16, cols = (c, u) u-inner
                # c = [ss_v 16 | vv_v 16 | sv (v,m) 48 | vs (v,m) 48]
                t_t = sbT.tile([P, 2048], BF16, tag="t_t")
                Zs = Z4[:, s * 128:(s + 1) * 128]
                # ss+vv: t[(g2,v,u)] = wb[(g2,v,u)] * z[(g2,u)]
                nc.vector.tensor_mul(
                    t_t[:, 0:512].rearrange("p (g v u) -> p g v u", g=2, u=16),
                    wb[:, 0:512].rearrange("p (g v u) -> p g v u", g=2, u=16),
                    Zs[:, 0:32].rearrange("p (g u) -> p g u", g=2)[:, :, None, :]
                        .to_broadcast([P, 2, 16, 16]))
                # sv: t[(v,m,u)] = wb_sv[(v,u)] * z_sv[(m,u)]
                nc.vector.tensor_mul(
                    t_t[:, 512:1280].rearrange("p (v m u) -> p v m u", m=3, u=16),
                    wb[:, None, 512:768].rearrange("p x (v u) -> p v x u", u=16)
                        .to_broadcast([P, 16, 3, 16]),
                    Zs[:, None, 32:80].rearrange("p x (m u) -> p x m u", u=16)
                        .to_broadcast([P, 16, 3, 16]))
                # vs: t[(v,m,u)] = wb_vs[(v,u)] * z_vs[(m,u)]
                nc.vector.tensor_mul(
                    t_t[:, 1280:2048].rearrange("p (v m u) -> p v m u", m=3, u=16),
                    wb[:, None, 768:1024].rearrange("p x (v u) -> p v x u", u=16)
                        .to_broadcast([P, 16, 3, 16]),
                    Zs[:, None, 80:128].rearrange("p x (m u) -> p x m u", u=16)
                        .to_broadcast([P, 16, 3, 16]))

                # ---- scatter: psO += Sel.T @ t  (4 x 512-col matmuls)
                if first[t_idx]:
                    sc_ps = psO.tile([P, 2048], F32, space="PSUM", tag="scps")
                    cur_blk = blk
                assert cur_blk == blk
                sel_s = sel4[:, s * P:(s + 1) * P]
                for c5 in range(4):
                    nc.tensor.matmul(sc_ps[:, c5 * 512:(c5 + 1) * 512],
                                     sel_s,
                                     t_t[:, c5 * 512:(c5 + 1) * 512],
                                     start=first[t_idx], stop=last[t_idx])

                if last[t_idx]:
                    # drain psO -> bf16, reduce over u, assemble, store
                    acc = sbO.tile([P, 2048], BF16, tag="acc")
                    nc.scalar.activation(acc[:, 0:1024], sc_ps[:, 0:1024],
                                         mybir.ActivationFunctionType.Copy)
                    nc.vector.tensor_copy(acc[:, 1024:2048], sc_ps[:, 1024:2048])
                    red = sbO.tile([P, 128], F32, tag="red")
                    nc.vector.reduce_sum(
                        red[:][:, :, None],
                        acc[:].rearrange("p (c u) -> p c u", u=16),
                        axis=mybir.AxisListType.X)
                    outb = sbO.tile([P, 64], F32, tag="outb")
                    nc.vector.tensor_add(outb[:, 0:16], red[:, 0:16], red[:, 16:32])
                    nc.vector.tensor_add(outb[:, 16:64], red[:, 32:80],
                                         red[:, 80:128])
                    nc.sync.dma_start(out_d[blk * P:(blk + 1) * P, :], outb[:])

    nc.finalize()
    return nc


def _prep(inputs):
    """Host-side sharding: bucket edges by (core, node-block), pad to a
    uniform per-block tile grid, gather source features, build per-core
    input maps. Pure data movement + dtype casts (no value arithmetic)."""
    nf = np.ascontiguousarray(inputs["node_features"], dtype=np.float32)
    esrc = inputs["edge_src"].astype(np.int64)
    edst = inputs["edge_dst"].astype(np.int64)
    eattr = np.asarray(inputs["edge_attr"], dtype=np.float32)
    eemb = np.asarray(inputs["edge_embedding"], dtype=np.float32)

    core = edst // NPC
    blk = (edst - core * NPC) // P
    key = core * NBLK + blk
    order = np.argsort(key, kind="stable")
    key_s = key[order]

    counts = np.bincount(key_s, minlength=NCORES * NBLK).reshape(NCORES, NBLK)
    tiles_per_block = [int(math.ceil(max(counts[:, b].max(), 1) / P))
                       for b in range(NBLK)]
    while sum(tiles_per_block) % 4:
        tiles_per_block[-1] += 1
    T = sum(tiles_per_block)
    E = T * P
    NG = T // 4
    starts = np.concatenate([[0], np.cumsum([t * P for t in tiles_per_block])])

    import ml_dtypes
    bf16 = ml_dtypes.bfloat16

    in_maps = []
    boundaries = np.searchsorted(key_s, np.arange(NCORES * NBLK + 1))
    for k in range(NCORES):
        emb_full = np.zeros((E, 64), np.float32)
        src_full = np.zeros((E, 64), np.float32)
        attr_full = np.zeros((E, 4), np.float32)
        dst_full = np.full((E,), 999.0, np.float32)
        for b in range(NBLK):
            lo, hi = boundaries[k * NBLK + b], boundaries[k * NBLK + b + 1]
            n = hi - lo
            sl = order[lo:hi]
            o = int(starts[b])
            emb_full[o:o + n] = eemb[sl]
            src_full[o:o + n] = nf[esrc[sl]]
            attr_full[o:o + n] = eattr[sl]
            dst_full[o:o + n] = (edst[sl] - k * NPC - b * P).astype(np.float32)
        embT = np.ascontiguousarray(emb_full.T.astype(bf16))
        # srcp [NG*128, 256]: subtile s of group g -> cols [64s:64(s+1)]
        srcp = np.ascontiguousarray(
            src_full.reshape(NG, 4, P, 64).transpose(0, 2, 1, 3)
            .reshape(NG * P, 256).astype(bf16))
        # attrp [NG*128, 32]: per subtile 8 cols [s2, v2x, v2y, v2z, dstr, 0..]
        ap8 = np.zeros((E, 8), np.float32)
        ap8[:, 0:4] = attr_full
        ap8[:, 4] = dst_full
        attrp = np.ascontiguousarray(
            ap8.reshape(NG, 4, P, 8).transpose(0, 2, 1, 3).reshape(NG * P, 32))
        in_maps.append(dict(embT=embT, srcp=srcp, attrp=attrp))
    return tiles_per_block, in_maps


def _w2_perm():
    """Permutation of W2's 1024 columns: (path,u,v) -> [ss|vv|sv|vs] each (v,u)."""
    idx = np.arange(1024).reshape(4, MUL, MUL)  # (path, u, v)
    blocks = [idx[0].T, idx[3].T, idx[2].T, idx[1].T]  # ss, vv, sv, vs as (v,u)
    return np.concatenate([b.reshape(-1) for b in blocks])


def _full_maps(inputs):
    import ml_dtypes
    bf16 = ml_dtypes.bfloat16
    tiles_per_block, in_maps = _prep(inputs)
    W1 = np.ascontiguousarray(np.asarray(inputs["W1"], np.float32).astype(bf16))
    b1 = np.ascontiguousarray(inputs["b1"], np.float32).reshape(HID, 1)
    W2p = np.ascontiguousarray(
        np.asarray(inputs["W2"], np.float32)[:, _w2_perm()].astype(bf16))
    iota = np.tile(np.arange(P, dtype=np.float32)[None, :], (P, 1))
    assert not np.any(inputs["b2"]), "b2 != 0 unsupported by this build"
    for m in in_maps:
        m.update(w1=W1, b1=b1, w2=W2p, iota=iota)
    return tiles_per_block, in_maps


def kernel(**inputs):
    tiles_per_block, in_maps = _full_maps(inputs)
    key = tuple(tiles_per_block)
    if key not in _CACHE:
        _CACHE[key] = _build(tiles_per_block)
    nc = _CACHE[key]

    res = bass_utils.run_bass_kernel_spmd(nc, in_maps,
                                          core_ids=list(range(NCORES)))
    out = np.empty((N_NODES, 64), np.float32)
    for k in range(NCORES):
        out[k * NPC:(k + 1) * NPC] = res.results[k]["out"][:NPC]
    return out
